# revision 37
# baseline (speedup 1.0000x reference)
"""Trainium2 Bass kernel for a single RoBERTa encoder layer.

Problem: B=8, S=512, H=1024, 16 heads (d=64), FF=4096, fp32 in/out, eval.

Strategy: data-parallel over batch (one batch element per core, 8 cores).
Per core, activations flow in a transposed "feature-on-partitions" layout.
Matmul dtypes: fp8e4 (e4m3) with DoubleRow perf mode (2x PE throughput)
for the QKV projections, the probs@V context matmul and the attention
output projection; bf16 (1 cyc/row) for scores, FFN1 and FFN2. Empirically
(see fp8 experiment) this config lands at ~7e-3 relative error vs the 2e-2
gate; fp8 in the FFN would exceed the budget.

Scaling tricks:
  - weights pre-scaled x16 before fp8 cast (avoids e4m3 subnormals),
    un-scaled for free via the ACT bias/scale path off PSUM.
  - exp() output scaled by exp(-2.34) so probs fit e4m3 nicely; softmax
    normalization (ones-rows trick inside the V'' matrix) cancels it.
  - host pre-transposes X and pre-packs all DoubleRow operand layouts.

Layout per core:
  xt (bf16, [h=128 x 8, tok 512])   transposed input, residual 1
  xt8 (fp8 pairs)                   QKV moving / V stationary operand
  qt/kt [feat 128, tok 512] bf16 -> scoresT[kpos, q] via 64-row stationary
  e8 = fp8(exp(scores/8 + mask - 2.34)) pairs -> ctx via DoubleRow with
    V''=[16(V+bv) | ones] -> ctx rows 0:64, sumexp rows 64:128
  pair8 = fp8(16*ctx) packed head pairs -> wo DoubleRow -> +bo +xt -> LN1
  FFN1 bf16 (wi stationary) -> gelu -> interT
  FFN2 bf16 activation-stationary (interT tiles) -> [tok, col] + aot^T
    + bo2 residual -> LN2 in normal layout -> bf16 out (host casts fp32)
"""
import math

import numpy as np
import ml_dtypes

import concourse.bass as bass
import concourse.mybir as mybir
import concourse.tile as tile
from concourse import bacc
from concourse import bass_utils
from concourse.masks import make_identity

F32 = mybir.dt.float32
BF16 = mybir.dt.bfloat16
FP8 = mybir.dt.float8e4
AF = mybir.ActivationFunctionType
ALU = mybir.AluOpType
DR = mybir.MatmulPerfMode.DoubleRow

B, S, H, NH, HD, FF = 8, 512, 1024, 16, 64, 4096
KT = H // 128       # 8 hidden k-tiles
PR = KT // 2        # 4 k-tile pairs (fp8 DoubleRow)
ST = S // 128       # 4 token tiles
FT = FF // 128      # 32 ff tiles
EPS = 1e-5
WS = 16.0           # weight prescale before fp8 cast
EXP_SHIFT = math.log(64.0) - 6.5   # folded into exp bias; cancels in softmax

_CACHE = {}


def _build():
    nc = bacc.Bacc("TRN2", target_bir_lowering=False, debug=False,
                   enable_asserts=True, num_devices=B)

    def din(name, shape, dt):
        return nc.dram_tensor(name, shape, dt, kind="ExternalInput").ap()

    # per-core inputs (host pre-packed)
    xt8_d = din("xt8", [512, 1024], FP8)        # pair-packed fp8 X^T
    xtb_d = din("xtb", [H, S], BF16)            # X^T bf16
    maskb_d = din("maskb", [128, ST], F32)      # per-kpos mask + EXP_SHIFT
    # shared weights
    wq8_d = din("wq8", [512, 2048], FP8)
    wk8_d = din("wk8", [512, 2048], FP8)
    wv8_d = din("wv8", [512, 2048], FP8)
    wo8_d = din("wo8", [512, 2048], FP8)
    wi_d = din("wi", [H, FF], BF16)
    wo2_d = din("wo2", [FF, H], BF16)
    # biast: bq bk bo g1 b1 (8 cols each) + bi (32) = 72 cols fp32
    biast_d = din("biast", [128, 72], F32)
    bv16_d = din("bv16", [1, H], BF16)          # 16*bv
    g1r_d = din("g1r", [1, H], BF16)
    b1bo2r_d = din("b1bo2r", [1, H], BF16)
    g2r_d = din("g2r", [1, H], BF16)
    b2r_d = din("b2r", [1, H], BF16)
    ones8_d = din("ones8", [128, 512], FP8)     # [1|0] / [0|1] sums stationary
    onec_d = din("onec", [128, 1], BF16)
    oner_d = din("oner", [1, 128], mybir.dt.float32r)
    out_d = nc.dram_tensor("out", [S, H], BF16, kind="ExternalOutput").ap()

    from contextlib import ExitStack
    es = ExitStack()
    with tile.TileContext(nc) as tc, es:
        # ---------- long-lived pools ----------
        cst = es.enter_context(tc.tile_pool(name="cst", bufs=1))
        p_aotbo = es.enter_context(tc.tile_pool(name="p_aotbo", bufs=4))
        es_xtb = ExitStack()
        p_xtb = es_xtb.enter_context(tc.tile_pool(name="p_xtb", bufs=1))
        p_wo8 = es_xtb.enter_context(tc.tile_pool(name="p_wo8", bufs=1))
        p_pair8 = es_xtb.enter_context(tc.tile_pool(name="p_pair8", bufs=4))
        es_qkv = ExitStack()
        p_xt8 = es_qkv.enter_context(tc.tile_pool(name="p_xt8", bufs=1))
        p_w8 = es_qkv.enter_context(tc.tile_pool(name="p_w8", bufs=3))
        p_vv8 = es_qkv.enter_context(tc.tile_pool(name="p_vv8", bufs=2))
        # right side: FFN pools (reserve order for LIFO closes)
        es_int = ExitStack()
        p_int = es_int.enter_context(tc.tile_pool(name="p_int", bufs=32, side="right"))
        es_aot = ExitStack()
        p_aot = es_aot.enter_context(tc.tile_pool(name="p_aot", bufs=8, side="right"))

        # ---------- DMA in (sync queue), consumption order ----------
        # combined single-DMA loads: [512, N] dram -> [128, 4, N] sbuf view
        def _load4(pool, dram, n, tag):
            big = pool.tile([128, 4 * n], FP8, tag=tag, name=f"big_{tag}")
            nc.sync.dma_start(
                out=big.rearrange("p (j n) -> p j n", j=4),
                in_=dram.rearrange("(j p) n -> p j n", p=128))
            return [big[:, n * j:n * (j + 1)] for j in range(4)]

        xt8 = _load4(p_xt8, xt8_d, 1024, "xt8")
        wv8 = _load4(p_w8, wv8_d, 2048, "wv8")
        t_biast = cst.tile([128, 72], F32, tag="t_biast")
        nc.sync.dma_start(out=t_biast, in_=biast_d)
        t_bq, t_bk = t_biast[:, 0:8], t_biast[:, 8:16]
        t_bo = t_biast[:, 16:24]
        t_g1, t_b1 = t_biast[:, 24:32], t_biast[:, 32:40]
        t_bi = t_biast[:, 40:72]
        t_mask = cst.tile([128, ST], F32, tag="t_mask")
        nc.sync.dma_start(out=t_mask, in_=maskb_d)
        t_onec = cst.tile([128, 1], BF16, tag="t_onec")
        nc.sync.dma_start(out=t_onec, in_=onec_d)
        t_oner = cst.tile([1, 128], mybir.dt.float32r, tag="t_oner")
        nc.sync.dma_start(out=t_oner, in_=oner_d)
        t_bv16 = cst.tile([128, H], BF16, tag="t_bv16")
        nc.sync.dma_start(out=t_bv16, in_=bv16_d.partition_broadcast(128))
        wq8 = _load4(p_w8, wq8_d, 2048, "wq8")
        wk8 = _load4(p_w8, wk8_d, 2048, "wk8")
        wo8 = _load4(p_wo8, wo8_d, 2048, "wo8")
        xtb_big = p_xtb.tile([128, KT * S], BF16, tag="xtb", name="xtb_big")
        nc.sync.dma_start(
            out=xtb_big.rearrange("p (j n) -> p j n", j=KT),
            in_=xtb_d.rearrange("(j p) n -> p j n", p=128))
        xtb = [xtb_big[:, S * j:S * (j + 1)] for j in range(KT)]
        t_g1r = cst.tile([128, H], BF16, tag="t_g1r")
        nc.sync.dma_start(out=t_g1r, in_=g1r_d.partition_broadcast(128))
        t_b1bo2r = cst.tile([128, H], BF16, tag="t_b1bo2r")
        nc.sync.dma_start(out=t_b1bo2r, in_=b1bo2r_d.partition_broadcast(128))
        t_g2r = cst.tile([128, H], BF16, tag="t_g2r")
        nc.sync.dma_start(out=t_g2r, in_=g2r_d.partition_broadcast(128))
        t_b2r = cst.tile([128, H], BF16, tag="t_b2r")
        nc.sync.dma_start(out=t_b2r, in_=b2r_d.partition_broadcast(128))

        ident = cst.tile([128, 128], BF16, tag="ident")
        make_identity(nc, ident)
        t_eps1 = cst.tile([1, 1], F32, tag="t_eps1")
        nc.vector.memset(t_eps1, EPS)
        t_eps128 = cst.tile([128, 1], F32, tag="t_eps128")
        nc.vector.memset(t_eps128, EPS)

        # V'' tiles: [128 kpos, i(2), pair(8), 256 = VA(64)|0(128)|VB(64)]
        vv8 = [p_vv8.tile([128, 4096], FP8, tag="vv8", name=f"vv8_{sp}")
               for sp in range(2)]
        for sp in range(2):
            nc.gpsimd.memset(vv8[sp], 0.0)
        t_ones8 = cst.tile([128, 512], FP8, tag="t_ones8")
        nc.sync.dma_start(out=t_ones8, in_=ones8_d)

        r3 = lambda t: t.rearrange("p (i n) -> p i n", i=2)

        # ---------- V projection (fp8 DoubleRow, activation-stationary) ----------
        with tc.tile_pool(name="ps_v", bufs=2, space="PSUM") as ps_v:
            for s in range(ST):
                for n in range(2):
                    ps = ps_v.tile([128, 512], F32, tag="psv")
                    for p in range(PR):
                        nc.tensor.matmul(
                            ps, r3(xt8[p])[:, :, 128 * s:128 * (s + 1)],
                            r3(wv8[p])[:, :, 512 * n:512 * (n + 1)],
                            start=(p == 0), stop=(p == PR - 1), perf_mode=DR)
                    vvw = vv8[s // 2].rearrange(
                        "p (i t c) -> p i t c", i=2, c=256)
                    psw = ps.rearrange("p (t two c) -> p t two c", two=2, c=64)
                    bvw = t_bv16[:, 512 * n:512 * (n + 1)].rearrange(
                        "p (t two c) -> p t two c", two=2, c=64)
                    nc.vector.tensor_tensor(
                        out=vvw[:, s % 2, 4 * n:4 * n + 4, 0:64],
                        in0=psw[:, :, 0, :], in1=bvw[:, :, 0, :], op=ALU.add)
                    nc.vector.tensor_tensor(
                        out=vvw[:, s % 2, 4 * n:4 * n + 4, 192:256],
                        in0=psw[:, :, 1, :], in1=bvw[:, :, 1, :], op=ALU.add)


        # ---------- Q/K projections (dense PE phase, biases on DVE) ----------
        pair8 = [p_pair8.tile([128, 1024], FP8, tag="pair8", name=f"pair8_{j}")
                 for j in range(PR)]
        # ---------- QKV + attention, software-pipelined ----------
        with tc.tile_pool(name="p_qt", bufs=3) as p_qt, \
             tc.tile_pool(name="p_kt", bufs=3) as p_kt:
            qt, kt = {}, {}
            with tc.tile_pool(name="ps_qk", bufs=2, space="PSUM") as ps_qk, \
                 tc.tile_pool(name="p_e8", bufs=4) as p_e8, \
                 tc.tile_pool(name="p_sums", bufs=3) as p_sums, \
                 tc.tile_pool(name="ps_sc", bufs=2, space="PSUM") as ps_sc, \
                 tc.tile_pool(name="ps_ctx", bufs=1, space="PSUM") as ps_ctx, \
                 tc.tile_pool(name="ps_pk", bufs=1, space="PSUM") as ps_pk:

                def emit_qk(t):
                    qt[t] = p_qt.tile([128, S], BF16, tag="qt", name=f"qt{t}")
                    ps_q = ps_qk.tile([128, S], F32, tag="psq")
                    for p in range(PR):
                        nc.tensor.matmul(
                            ps_q, r3(wq8[p])[:, :, 128 * t:128 * (t + 1)],
                            r3(xt8[p]), start=(p == 0), stop=(p == PR - 1),
                            perf_mode=DR)
                    nc.scalar.activation(out=qt[t], in_=ps_q, func=AF.Identity,
                                         bias=t_bq[:, t:t + 1], scale=1.0 / WS)
                    kt[t] = p_kt.tile([128, S], BF16, tag="kt", name=f"kt{t}")
                    ps_k = ps_qk.tile([128, S], F32, tag="psq")
                    for p in range(PR):
                        nc.tensor.matmul(
                            ps_k, r3(wk8[p])[:, :, 128 * t:128 * (t + 1)],
                            r3(xt8[p]), start=(p == 0), stop=(p == PR - 1),
                            perf_mode=DR)
                    nc.scalar.activation(out=kt[t], in_=ps_k, func=AF.Identity,
                                         bias=t_bk[:, t:t + 1], scale=1.0 / WS)

                def emit_scores(t):
                    # e8c: [kpos, i(kt pair), qA | qB]
                    e8c = [p_e8.tile([128, 2048], FP8, tag="e8",
                                     name=f"e8_{t}_{_i}") for _i in range(2)]
                    for kt_i in range(ST):
                        psc = ps_sc.tile([128, 1024], F32, tag="sc")
                        for hh in range(2):
                            lo, hi = 64 * hh, 64 * hh + 64
                            nc.tensor.matmul(
                                psc[:, 512 * hh:512 * (hh + 1)],
                                kt[t][lo:hi, 128 * kt_i:128 * (kt_i + 1)],
                                qt[t][lo:hi, :], start=True, stop=True)
                        nc.scalar.activation(
                            out=e8c[kt_i // 2].rearrange(
                                "p (i n) -> p i n", i=2)[:, kt_i % 2, :],
                            in_=psc, func=AF.Exp,
                            bias=t_mask[:, kt_i:kt_i + 1], scale=1.0 / 8.0)
                    return e8c

                ones8r = t_ones8.rearrange("p (g i c) -> p g i c", g=2, c=128)

                def emit_ctx(t, e8c):
                    # heads A/B stacked in one [128,512] psum via zero-padded
                    # stationaries [VA|0] / [0|VB]; sums likewise with [1|0]/[0|1]
                    ps_c = ps_ctx.tile([128, S], F32, tag="ctx")
                    ps_s2 = ps_pk.tile([128, S], F32, tag="pk")
                    k = 0
                    for hh in range(2):
                        for sp in range(2):
                            mv = e8c[sp].rearrange(
                                "p (i g n) -> p i g n", i=2, n=512)[:, :, hh, :]
                            st = vv8[sp].rearrange(
                                "p (i t c) -> p i t c", i=2, c=256)[
                                    :, :, t, 128 * hh:128 * hh + 128]
                            nc.tensor.matmul(ps_c, st, mv, start=(k == 0),
                                             stop=(k == 3), perf_mode=DR)
                            nc.tensor.matmul(ps_s2, ones8r[:, hh, :, :], mv,
                                             start=(k == 0), stop=(k == 3),
                                             perf_mode=DR)
                            k += 1
                    sums = p_sums.tile([128, S], F32, tag="sums")
                    nc.vector.tensor_copy(out=sums, in_=ps_s2)
                    rinv = p_sums.tile([128, S], F32, tag="sums")
                    nc.vector.reciprocal_approx_fast(out=rinv, in_=sums)
                    nc.vector.tensor_tensor(
                        out=pair8[t // 2][:, 512 * (t % 2):512 * (t % 2 + 1)],
                        in0=ps_c, in1=rinv, op=ALU.mult)

                emit_qk(0)
                emit_qk(1)
                e8c_prev = emit_scores(0)
                for t in range(1, KT):
                    if t + 1 < KT:
                        emit_qk(t + 1)
                    e8c_cur = emit_scores(t)
                    emit_ctx(t - 1, e8c_prev)
                    e8c_prev = e8c_cur
                emit_ctx(KT - 1, e8c_prev)
        es_qkv.close()

        # ---------- attention output proj + residual + LN1 ----------
        zt = []
        aotbo = [p_aotbo.tile([128, H], BF16, tag="aotbo", name=f"aotbo{s}")
                 for s in range(ST)]
        with tc.tile_pool(name="p_ztn", bufs=4) as p_ztn, \
             tc.tile_pool(name="p_mrst", bufs=8) as p_mrst, \
             tc.tile_pool(name="p_ab", bufs=2) as p_ab, \
             tc.tile_pool(name="p_zt", bufs=8) as p_zt, \
             tc.tile_pool(name="p_ztmp", bufs=3) as p_ztmp, \
             tc.tile_pool(name="p_sq", bufs=3) as p_sq, \
             tc.tile_pool(name="p_stat", bufs=1) as p_stat, \
             tc.tile_pool(name="p_rep", bufs=1) as p_rep, \
             tc.tile_pool(name="ps_wo", bufs=2, space="PSUM") as ps_wo, \
             tc.tile_pool(name="ps_stat", bufs=2, space="PSUM") as ps_stat, \
             tc.tile_pool(name="ps_tp", bufs=1, space="PSUM") as ps_tp, \
             tc.tile_pool(name="ps_rep", bufs=1, space="PSUM") as ps_rep:
            ztn = [p_ztn.tile([128, H], BF16, tag="ztn", name=f"ztn{s}")
                   for s in range(ST)]
            ps_sum = ps_stat.tile([1, S], F32, tag="lnsum")
            ps_sumsq = ps_stat.tile([1, S], F32, tag="lnsum")
            for m in range(KT):
                ps = ps_wo.tile([128, S], F32, tag="wo")
                for p in range(PR):
                    nc.tensor.matmul(ps, r3(wo8[p])[:, :, 128 * m:128 * (m + 1)],
                                     r3(pair8[p]), start=(p == 0),
                                     stop=(p == PR - 1), perf_mode=DR)
                ztmp = p_ztmp.tile([128, S], BF16, tag="ztmp")
                nc.vector.tensor_scalar(out=ztmp, in0=ps,
                                        scalar1=1.0 / (WS * WS),
                                        scalar2=t_bo[:, m:m + 1],
                                        op0=ALU.mult, op1=ALU.add)
                z = p_zt.tile([128, S], BF16, tag="zt", name=f"zt{m}")
                nc.vector.tensor_tensor(out=z, in0=ztmp, in1=xtb[m], op=ALU.add)
                zt.append(z)
                sq = p_sq.tile([128, S], BF16, tag="sq")
                nc.vector.tensor_tensor(out=sq, in0=z, in1=z, op=ALU.mult)
                nc.tensor.matmul(ps_sum, t_onec, z, start=(m == 0),
                                 stop=(m == KT - 1))
                nc.tensor.matmul(ps_sumsq, t_onec, sq, start=(m == 0),
                                 stop=(m == KT - 1))
            F32R = mybir.dt.float32r
            mu_f = p_stat.tile([1, S], F32R, tag="mu_f")
            nc.vector.tensor_scalar(out=mu_f, in0=ps_sum, scalar1=1.0 / H,
                                    scalar2=None, op0=ALU.mult)
            ex2 = p_stat.tile([1, S], F32, tag="ex2")
            nc.vector.tensor_scalar(out=ex2, in0=ps_sumsq, scalar1=1.0 / H,
                                    scalar2=None, op0=ALU.mult)
            mu2 = p_stat.tile([1, S], F32, tag="mu2")
            nc.vector.tensor_tensor(out=mu2, in0=mu_f, in1=mu_f, op=ALU.mult)
            var = p_stat.tile([1, S], F32, tag="var")
            nc.vector.tensor_tensor(out=var, in0=ex2, in1=mu2, op=ALU.subtract)
            sd = p_stat.tile([1, S], F32, tag="sd")
            nc.scalar.activation(out=sd, in_=var, func=AF.Sqrt, bias=t_eps1,
                                 scale=1.0)
            rstd_f = p_stat.tile([1, S], F32, tag="rstd_f")
            nc.vector.reciprocal_approx_fast(out=rstd_f, in_=sd)
            rstd_r = p_stat.tile([1, S], F32R, tag="rstd_r")
            nc.scalar.activation(out=rstd_r, in_=rstd_f, func=AF.Identity)
            ps_rep2 = ps_rep.tile([128, 2 * S], F32, tag="murep")
            nc.tensor.matmul(ps_rep2[:, 0:S], t_oner, mu_f,
                             start=True, stop=True)
            nc.tensor.matmul(ps_rep2[:, S:2 * S], t_oner, rstd_r,
                             start=True, stop=True)
            rep_big = p_rep.tile([128, 2 * S], BF16, tag="murep_sb")
            nc.vector.tensor_copy(out=rep_big, in_=ps_rep2)
            murep, rstdrep = rep_big[:, 0:S], rep_big[:, S:2 * S]
            aot = [p_aot.tile([128, S], BF16, tag="aot", name=f"aot{m}")
                   for m in range(KT)]
            for m in range(KT):
                t1 = p_sq.tile([128, S], BF16, tag="t1")
                nc.vector.tensor_tensor(out=t1, in0=zt[m], in1=murep,
                                        op=ALU.subtract)
                t2 = p_sq.tile([128, S], BF16, tag="t2")
                nc.vector.tensor_tensor(out=t2, in0=t1, in1=rstdrep, op=ALU.mult)
                nc.vector.tensor_scalar(out=aot[m], in0=t2,
                                        scalar1=t_g1[:, m:m + 1],
                                        scalar2=t_b1[:, m:m + 1],
                                        op0=ALU.mult, op1=ALU.add)
            # zt transposes (PE work that runs during the LN1 scalar chain)
            for m in range(KT):
                for s in range(ST):
                    pt = ps_tp.tile([128, 128], BF16, tag="tp")
                    nc.tensor.transpose(pt, zt[m][:, 128 * s:128 * (s + 1)],
                                        ident)
                    nc.vector.tensor_copy(
                        out=ztn[s][:, 128 * m:128 * (m + 1)], in_=pt)
            # per-token mu/rstd columns via tiny bf16 transposes
            mrb = p_stat.tile([1, 2 * S], BF16, tag="mrb")
            nc.vector.tensor_copy(out=mrb[:, 0:S], in_=mu_f)
            nc.vector.tensor_copy(out=mrb[:, S:2 * S], in_=rstd_f)
            for s in range(ST):
                ptm = ps_tp.tile([128, 1], BF16, tag="tpm", name=f"ptm{s}")
                nc.tensor.transpose(ptm, mrb[:, 128 * s:128 * (s + 1)],
                                    ident[0:1, 0:1])
                ptr_ = ps_tp.tile([128, 1], BF16, tag="tpm", name=f"ptr{s}")
                nc.tensor.transpose(ptr_, mrb[:, S + 128 * s:S + 128 * (s + 1)],
                                    ident[0:1, 0:1])
                mut = p_mrst.tile([128, 1], F32, tag="mut", name=f"mut{s}")
                nc.vector.tensor_copy(out=mut, in_=ptm)
                rstdt = p_mrst.tile([128, 1], F32, tag="rstdt", name=f"rst{s}")
                nc.vector.tensor_copy(out=rstdt, in_=ptr_)
                nmneg = p_mrst.tile([128, 1], F32, tag="nmneg", name=f"nm{s}")
                nc.vector.tensor_scalar(out=nmneg, in0=mut, scalar1=rstdt,
                                        scalar2=-1.0, op0=ALU.mult,
                                        op1=ALU.mult)
                # aotbo[s] = (ztn - mu)*rstd*g1 + (b1 + bo2), normal layout
                ya = p_ab.tile([128, H], BF16, tag="ya")
                nc.scalar.activation(out=ya, in_=ztn[s], func=AF.Identity,
                                     bias=nmneg, scale=rstdt)
                yb = p_ab.tile([128, H], BF16, tag="yb")
                nc.vector.tensor_tensor(out=yb, in0=ya, in1=t_g1r, op=ALU.mult)
                nc.vector.tensor_tensor(out=aotbo[s], in0=yb, in1=t_b1bo2r,
                                        op=ALU.add)
        es_xtb.close()

        # wo2 resident: lands in the freed attention/xtb space
        p_wo2r = es.enter_context(tc.tile_pool(name="p_wo2r", bufs=1))
        wo2r_big = p_wo2r.tile([128, FT * H], BF16, tag="wo2r", name="wo2r_big")
        nc.gpsimd.dma_start(
            out=wo2r_big.rearrange("p (j n) -> p j n", j=FT),
            in_=wo2_d.rearrange("(j p) n -> p j n", p=128))
        wo2r = [wo2r_big[:, H * f:H * (f + 1)] for f in range(FT)]

        # ---------- FFN1 (bf16, wi stationary) ----------
        intert = [p_int.tile([128, S], BF16, tag="intert", name=f"int{f}")
                  for f in range(FT)]
        with tc.tile_pool(name="p_wi", bufs=32) as p_wi, \
             tc.tile_pool(name="ps_f1", bufs=3, space="PSUM") as ps_f1:
            for fb in range(FT // 4):
                wic = []
                for k in range(KT):
                    c = p_wi.tile([128, 512], BF16, tag="wi")
                    nc.sync.dma_start(
                        out=c, in_=wi_d[128 * k:128 * (k + 1),
                                        512 * fb:512 * (fb + 1)])
                    wic.append(c)
                for fi in range(4):
                    f = 4 * fb + fi
                    ps = ps_f1.tile([128, S], F32, tag="f1")
                    for k in range(KT):
                        nc.tensor.matmul(ps, wic[k][:, 128 * fi:128 * (fi + 1)],
                                         aot[k], start=(k == 0),
                                         stop=(k == KT - 1))
                    nc.scalar.activation(out=intert[f], in_=ps, func=AF.Gelu,
                                         bias=t_bi[:, f:f + 1], scale=1.0)
        es_aot.close()

        # ---------- FFN2 (bf16, activation-stationary, s-outer) + LN2 ----------
        with tc.tile_pool(name="p_z2", bufs=3) as p_z2, \
             tc.tile_pool(name="p_sq2", bufs=2) as p_sq2, \
             tc.tile_pool(name="p_st2", bufs=4) as p_st2, \
             tc.tile_pool(name="p_y", bufs=3) as p_y, \
             tc.tile_pool(name="ps_f2", bufs=4, space="PSUM") as ps_f2:
          if True:
            for s in range(ST):
                ps_a = ps_f2.tile([128, 512], F32, tag="f2")
                ps_b = ps_f2.tile([128, 512], F32, tag="f2")
                for f in range(FT):
                    lhsT = intert[f][:, 128 * s:128 * (s + 1)]
                    nc.tensor.matmul(ps_a, lhsT, wo2r[f][:, 0:512],
                                     start=(f == 0), stop=(f == FT - 1))
                    nc.tensor.matmul(ps_b, lhsT, wo2r[f][:, 512:1024],
                                     start=(f == 0), stop=(f == FT - 1))
                z2 = p_z2.tile([128, H], BF16, tag="z2")
                for hh, psx in ((0, ps_a), (1, ps_b)):
                    nc.vector.tensor_tensor(
                        out=z2[:, 512 * hh:512 * (hh + 1)], in0=psx,
                        in1=aotbo[s][:, 512 * hh:512 * (hh + 1)], op=ALU.add)
                s1 = p_st2.tile([128, 1], F32, tag="s1")
                nc.vector.tensor_reduce(out=s1, in_=z2, axis=mybir.AxisListType.X,
                                        op=ALU.add)
                sq2 = p_sq2.tile([128, H], BF16, tag="sq2")
                s2 = p_st2.tile([128, 1], F32, tag="s2")
                nc.scalar.activation(out=sq2, in_=z2, func=AF.Square,
                                     accum_out=s2)
                mu = p_st2.tile([128, 1], F32, tag="mu")
                nc.vector.tensor_scalar(out=mu, in0=s1, scalar1=1.0 / H,
                                        scalar2=None, op0=ALU.mult)
                ex2b = p_st2.tile([128, 1], F32, tag="ex2b")
                nc.vector.tensor_scalar(out=ex2b, in0=s2, scalar1=1.0 / H,
                                        scalar2=None, op0=ALU.mult)
                mu2b = p_st2.tile([128, 1], F32, tag="mu2b")
                nc.vector.tensor_tensor(out=mu2b, in0=mu, in1=mu, op=ALU.mult)
                varb = p_st2.tile([128, 1], F32, tag="varb")
                nc.vector.tensor_tensor(out=varb, in0=ex2b, in1=mu2b,
                                        op=ALU.subtract)
                sdb = p_st2.tile([128, 1], F32, tag="sdb")
                nc.scalar.activation(out=sdb, in_=varb, func=AF.Sqrt,
                                     bias=t_eps128, scale=1.0)
                rstdb = p_st2.tile([128, 1], F32, tag="rstdb")
                nc.vector.reciprocal(out=rstdb, in_=sdb)
                nmu = p_st2.tile([128, 1], F32, tag="nmu")
                nc.vector.tensor_tensor(out=nmu, in0=mu, in1=rstdb, op=ALU.mult)
                nb = p_st2.tile([128, 1], F32, tag="nb")
                nc.vector.tensor_scalar(out=nb, in0=nmu, scalar1=-1.0,
                                        scalar2=None, op0=ALU.mult)
                y1 = p_y.tile([128, H], BF16, tag="y1")
                nc.scalar.activation(out=y1, in_=z2, func=AF.Identity,
                                     bias=nb, scale=rstdb)
                y2 = p_y.tile([128, H], BF16, tag="y2")
                nc.vector.tensor_tensor(out=y2, in0=y1, in1=t_g2r, op=ALU.mult)
                y3 = p_y.tile([128, H], BF16, tag="y3")
                nc.vector.tensor_tensor(out=y3, in0=y2, in1=t_b2r, op=ALU.add)
                nc.sync.dma_start(out=out_d[128 * s:128 * (s + 1), :], in_=y3)
        es_int.close()

    nc.compile()
    return nc


def _get_nc():
    if "nc" not in _CACHE:
        _CACHE["nc"] = _build()
    return _CACHE["nc"]


def _perpart(v):
    # [n*128] -> [128, n] with vT[p, t] = v[t*128 + p]
    v = np.asarray(v, np.float32)
    return np.ascontiguousarray(v.reshape(-1, 128).T)


def _pack_pairs(w, scale, dtype):
    # w [K, N] -> [K/2, 2N] fp8: row (128p + k) holds [i in {0,1}, n] with
    # value w[(2p+i)*128 + k, n] * scale
    w = np.asarray(w, np.float32) * scale
    K, N = w.shape
    t = w.reshape(K // 256, 2, 128, N).transpose(0, 2, 1, 3)
    return np.ascontiguousarray(t.reshape(K // 2, 2 * N)).astype(dtype)


def _ones8():
    # [128, (g, i, c)]: g=0: cols 0:64 ones; g=1: cols 64:128 ones
    a = np.zeros((128, 2, 2, 128), np.float32)
    a[:, 0, :, 0:64] = 1.0
    a[:, 1, :, 64:128] = 1.0
    return a.reshape(128, 512).astype(ml_dtypes.float8_e4m3)


def _shared_inputs(inp):
    f8 = ml_dtypes.float8_e4m3
    bf = ml_dtypes.bfloat16
    f = np.float32
    biast = np.concatenate(
        [_perpart(inp["bq"]), _perpart(inp["bk"]), _perpart(inp["bo"]),
         _perpart(inp["ln1_g"]), _perpart(inp["ln1_b"]), _perpart(inp["bi"])],
        axis=1).astype(f)
    return {
        "wq8": _pack_pairs(inp["wq"], WS, f8),
        "wk8": _pack_pairs(inp["wk"], WS, f8),
        "wv8": _pack_pairs(inp["wv"], WS, f8),
        "wo8": _pack_pairs(inp["wo"], WS, f8),
        "wi": np.ascontiguousarray(np.asarray(inp["wi"], f)).astype(bf),
        "wo2": np.ascontiguousarray(np.asarray(inp["wo2"], f)).astype(bf),
        "biast": biast,
        "bv16": (np.asarray(inp["bv"], f) * WS).reshape(1, H).astype(bf),
        "g1r": np.asarray(inp["ln1_g"], f).reshape(1, H).astype(bf),
        "b1bo2r": (np.asarray(inp["ln1_b"], f)
                   + np.asarray(inp["bo2"], f)).reshape(1, H).astype(bf),
        "g2r": np.asarray(inp["ln2_g"], f).reshape(1, H).astype(bf),
        "b2r": np.asarray(inp["ln2_b"], f).reshape(1, H).astype(bf),
        "ones8": _ones8(),
        "onec": np.ones((128, 1), f).astype(bf),
        "oner": np.ones((1, 128), f),
    }


def _core_inputs(shared, hs, am, b):
    f8 = ml_dtypes.float8_e4m3
    bf = ml_dtypes.bfloat16
    xT = np.ascontiguousarray(hs[b].T)  # [H, S] fp32
    return dict(
        shared,
        xt8=_pack_pairs(xT, 1.0, f8),
        xtb=xT.astype(bf),
        maskb=(_perpart(am[b]) + EXP_SHIFT).astype(np.float32),
    )


def kernel(hidden_states, attention_mask, wq, bq, wk, bk, wv, bv,
           wo, bo, ln1_g, ln1_b, wi, bi, wo2, bo2, ln2_g, ln2_b):
    nc = _get_nc()
    f = np.float32
    shared = _shared_inputs({
        "wq": wq, "wk": wk, "wv": wv, "wo": wo, "wi": wi, "wo2": wo2,
        "bq": bq, "bk": bk, "bv": bv, "bo": bo, "bi": bi, "bo2": bo2,
        "ln1_g": ln1_g, "ln1_b": ln1_b, "ln2_g": ln2_g, "ln2_b": ln2_b,
    })
    hs = np.ascontiguousarray(hidden_states, f)
    am = np.ascontiguousarray(attention_mask, f).reshape(B, S)
    in_maps = [_core_inputs(shared, hs, am, b) for b in range(B)]
    res = bass_utils.run_bass_kernel_spmd(nc, in_maps, core_ids=list(range(B)),
                                          trace=False)
    return np.stack([res.results[b]["out"].astype(f) for b in range(B)])
